# revision 98
# baseline (speedup 1.0000x reference)
"""Attentional pooling layer on Trainium2 (Bass/Tile), 8-core batch-parallel.

Reference computation per batch b:
    scores[hw, n] = sum_c f[c, hw] * w[c, n]          (mm1)
    num           = softplus(scores)                  (ACT: Abs/Exp/Ln)
    denom[n]      = sum_hw num[hw, n]                 (PE reduce)
    att[hw, n]    = num / denom[n]                    (DVE recip + mult)
    out[c, n]     = sum_hw f[c, hw] * att[hw, n]      (mm2)

HBM time is the bottleneck, so bytes are minimized against the rel-err
budget (2e-2): weights travel as fp8 e3m4 (see WS below), activations and
outputs as bf16.  Measured end-to-end rel err: 1.49e-2 (seed-fixed,
deterministic).  The +1e-4 numerator / +16e-4 denominator constants of the
reference are dropped entirely (measured end-to-end impact 5.5e-5).

Partition layout: 4 batches per group at 32-partition offsets 0/32/64/96
(explicit tile_position, bypassing the conservative 0/32/64 helper assert;
base 96 verified on hardware).  32 batches per core = 8 exact groups.

The denominator reduce-over-hw AND its broadcast back to all 128
partitions are fused into ONE matmul with a constant 0/1 matrix
C[k, m] = 1 iff k, m in the same 32-block and k%32 < 16.  A single act
table (natural_log_exp_and_others: Abs+Exp+Ln+Copy) serves every ACT op,
so exactly one table load is emitted — forced by blanking the other table
entries (indices preserved, so act_func_set_id still matches walrus's
act_info.json).

Output drain PSUM->SBUF casts f32->bf16 on DVE/ACT (GPSIMD cannot touch
PSUM), in [128,1024] units (2 PSUM banks) to amortize access latency.
"""

import numpy as np
import ml_dtypes
from contextlib import ExitStack

import concourse.bass as bass
import concourse.bacc as bacc
import concourse.tile as tile
from concourse import mybir
from concourse.bass_utils import run_bass_kernel_spmd

F32 = mybir.dt.float32
BF16 = mybir.dt.bfloat16
F8E3 = mybir.dt.float8e3
AF = mybir.ActivationFunctionType
ALU = mybir.AluOpType
BF_NP = ml_dtypes.bfloat16

# Weights travel as fp8 e3m4 (1 byte), halving the weight-load traffic again.
# The PE multiplies fp8 operands exactly, so only the quantization of w
# matters: end-to-end rel err 1.50e-2 vs the 2e-2 gate (measured, seed-fixed,
# deterministic).  w is pre-scaled by WS=2.5 into e3m4's [~0.008, 15.5]
# band; features carry 1/WS so scores come out exact, and the denominator
# matrix carries 1/WS so mm2's (f/WS) @ (WS*att) cancels exactly.
WS = 2.5

# Force every ACT op onto one table so only a single table load is emitted.
# Keys/positions are preserved (ids index walrus's act_info.json); only the
# *selection* changes: all other tables are presented as empty so the greedy
# insert pass picks natural_log_exp_and_others (Abs+Exp+Ln+Copy) for all.
_KEEP_TABLE = "natural_log_exp_and_others"
_orig_get_tables = bacc.get_activation_tables


def _single_table(arch):
    tbls = _orig_get_tables(arch)
    return {k: (v if k == _KEEP_TABLE else set()) for k, v in tbls.items()}


bacc.get_activation_tables = _single_table

N_CORES = 8
B_FULL, C, H, W, N = 256, 256, 4, 4, 2048
HW = H * W                  # 16
B = B_FULL // N_CORES       # 32 batches per core
KC = C // 128               # 2 contraction chunks of 128
GB = 4                      # batches per partition group (offsets 0/32/64/96)
GP = 32 * GB                # 128 partitions per group
NCH = 4                     # n chunks per group
NW = N // NCH               # 512 (one PSUM bank)


def aux_inputs():
    # aux[:, 0, :] = cmat: cmat[k, m] = 1 iff k and m share a 32-block and
    # k%32 is a real hw row; (C^T @ num) both reduces over hw and broadcasts
    # back to 128 partitions.  aux[:, 1, :] = 128x128 identity (transposes).
    cmat = np.zeros((GP, GP), np.float32)
    for k in range(GP):
        for m in range(GP):
            if k // 32 == m // 32 and k % 32 < HW:
                cmat[k, m] = 1.0 / WS
    aux = np.stack([cmat, np.eye(128, dtype=np.float32)], axis=1)
    return {"aux": aux.astype(BF_NP)}


def build_nc(n_batch=B, debug=False):
    nc = bacc.Bacc(None, target_bir_lowering=False, debug=debug)
    feat = nc.dram_tensor("fpad", [128, KC, n_batch, 32], BF16, kind="ExternalInput")
    wts = nc.dram_tensor("weights", [n_batch, C, N], F8E3, kind="ExternalInput")
    out = nc.dram_tensor("out", [n_batch, C, N], BF16, kind="ExternalOutput")
    aux_d = nc.dram_tensor("aux", [128, 2, 128], BF16, kind="ExternalInput")

    # [ci, b, kc, n] views of the DRAM tensors
    wts_r = wts.ap().rearrange("b (kc ci) n -> ci b kc n", kc=KC)
    out_r = out.ap().rearrange("b (kc ci) n -> ci b kc n", kc=KC)

    n_groups = (n_batch + GB - 1) // GB

    with tile.TileContext(nc) as tc, ExitStack() as ctx:
        singles = ctx.enter_context(tc.tile_pool(name="singles", bufs=1))
        wpool = ctx.enter_context(tc.tile_pool(name="w", bufs=18))
        opool = ctx.enter_context(tc.tile_pool(name="o", bufs=6))
        stashpool = ctx.enter_context(tc.tile_pool(name="stash", bufs=44))
        stash = []
        late_sp = []
        gate_inst = None
        gate2_inst = None
        first_mm1 = {}
        sppool = ctx.enter_context(tc.tile_pool(name="sp", bufs=2))
        numpool = ctx.enter_context(tc.tile_pool(name="num", bufs=2))
        rpool = ctx.enter_context(tc.tile_pool(name="r", bufs=2))
        attpool = ctx.enter_context(tc.tile_pool(name="att", bufs=2))
        ftpool = ctx.enter_context(tc.tile_pool(name="ft", bufs=2))
        ps_sc = ctx.enter_context(tc.tile_pool(name="ps_sc", bufs=2, space="PSUM"))
        ps_d = ctx.enter_context(tc.tile_pool(name="ps_d", bufs=1, space="PSUM"))
        ps_ft = ctx.enter_context(tc.tile_pool(name="ps_ft", bufs=1, space="PSUM"))
        ps_o = ctx.enter_context(tc.tile_pool(name="ps_o", bufs=2, space="PSUM"))

        # features, pre-transposed + hw-padded to 32 with zeros on the host
        f_t = singles.tile([128, KC, n_batch, 32], BF16)
        nc.sync.dma_start(out=f_t, in_=feat.ap())

        aux_t = singles.tile([128, 2, 128], BF16)
        nc.sync.dma_start(out=aux_t, in_=aux_d.ap())
        cm_t = aux_t[:, 0, :]
        id_t = aux_t[:, 1, :]

        def group_bs(gg):
            return [min(gg * GB + j, n_batch - 1) for j in range(GB)]

        # W loads are issued two groups ahead of use, so on the SP queue they
        # sit in front of the piece stores of the group being computed —
        # otherwise those stores' copy-waits head-of-line-block ready loads
        # and starve the DMA engines
        w_t = {}

        def load_w(b):
            if b not in w_t:
                w_t[b] = wpool.tile([128, KC, N], F8E3, tag="w", name="w_t")
                nc.sync.dma_start(out=w_t[b], in_=wts_r[:, b])

        def load_group_w(gg):
            if gg < n_groups:
                for b in sorted(set(group_bs(gg))):
                    load_w(b)

        load_group_w(0)
        load_group_w(1)
        load_group_w(2)
        load_group_w(3)

        ev = 0
        prev_dve_copies = []
        prev_act_copies = []
        for g in range(n_groups):
            bs = group_bs(g)
            emit = [g * GB + j < n_batch for j in range(GB)]
            dve_copies, act_copies = [], []

            # transposed features fT[hw, c] for mm2, one 128x128 transpose
            # per kc (slot j's fT lands at partition 32j automatically)
            ft_ps = ps_ft.tile([GP, KC, 128], BF16, name="ft_ps")
            for kc in range(KC):
                nc.tensor.transpose(
                    ft_ps[:, kc, :],
                    f_t[:, kc, bs[0] : bs[0] + GB, :],
                    id_t,
                )
            ft_sb = ftpool.tile([GP, KC, 128], BF16, name="ft_sb")
            nc.scalar.copy(ft_sb, ft_ps)

            # two halves of 2 n-chunks each: mm2 for a half only needs that
            # half's att chunks, so output stores stream mid-group instead of
            # bunching at the group end (shrinks the tail and group-boundary
            # DMA droughts)
            for h in range(2):
                # issue 2 of group g+2's weight loads at the top of each
                # half: the SP queue then alternates [2 loads, ~7 stores],
                # and each half's chunk-compute lull is covered by the loads
                # issued just ahead of the stores
                if g + 4 < n_groups:
                    bs2 = group_bs(g + 4)
                    load_w(bs2[2 * h])
                att_h = attpool.tile([GP, 2, NW], BF16, tag="att")
                for q in range(2):
                    nb = 2 * h + q
                    sc_ps = ps_sc.tile([GP, NW], F32, name="sc_ps")
                    for j in range(GB):
                        for kc in range(KC):
                            mm = nc.tensor.matmul(
                                sc_ps[32 * j : 32 * j + 32, :],
                                f_t[:, kc, bs[j], :],
                                w_t[bs[j]][:, kc, nb * NW : (nb + 1) * NW],
                                start=(kc == 0),
                                stop=(kc == KC - 1),
                                tile_position=(0, 32 * j),
                            )
                            if g == n_groups - 1 and gate_inst is None:
                                gate_inst = mm
                            if g == 1 and h == 1 and gate2_inst is None:
                                gate2_inst = mm
                            if h == 0 and g not in first_mm1:
                                first_mm1[g] = mm
                    # softplus(x) = max(x,0) + ln(1 + exp(-|x|)): exp arg <= 0
                    # so no overflow, Ln input stays in [1,2]
                    t_abs = sppool.tile([GP, NW], F32, tag="tabs")
                    nc.scalar.activation(t_abs, sc_ps, AF.Abs)
                    t_exp = sppool.tile([GP, NW], F32, tag="texp")
                    nc.scalar.activation(t_exp, t_abs, AF.Exp, scale=-1.0)
                    t_ln = sppool.tile([GP, NW], F32, tag="tln")
                    nc.scalar.activation(t_ln, t_exp, AF.Ln, bias=1.0)
                    num_t = numpool.tile([GP, NW], BF16, tag="num")
                    nc.vector.scalar_tensor_tensor(
                        num_t, sc_ps, 0.0, t_ln, op0=ALU.max, op1=ALU.add
                    )
                    # fused reduce-over-hw + broadcast:
                    #   d[m] = sum_k C[k,m] num[k]
                    d_ps = ps_d.tile([GP, NW], F32, name="d_ps")
                    nc.tensor.matmul(d_ps, cm_t, num_t, start=True, stop=True)
                    r_t = rpool.tile([GP, NW], F32, tag="r")
                    nc.vector.reciprocal(r_t, d_ps)
                    # att = num * (1/d): all-SBUF, so it can run on the
                    # otherwise-idle GPSIMD engine instead of DVE
                    nc.gpsimd.tensor_tensor(
                        att_h[:, q, :], num_t, r_t, op=ALU.mult
                    )

                for j in range(GB):
                    if not emit[j]:
                        continue
                    for kc in range(KC):
                        o_ps = ps_o.tile([128, 2, NW], F32)
                        for q in range(2):
                            nc.tensor.matmul(
                                o_ps[:, q, :],
                                ft_sb[32 * j : 32 * j + HW, kc, :],
                                att_h[32 * j : 32 * j + HW, q, :],
                                start=True,
                                stop=True,
                                tile_position=(32 * j, 0),
                            )
                        # stash some pieces: their DMAs are deferred into DMA
                        # lulls.  late wave -> the tail; early wave -> the
                        # t~50us load-burst lull; mid waves -> the recurring
                        # group-boundary lulls two groups later
                        late = (
                            g < n_groups - 1 and kc == 0 and (j == 0 or (j == 1 and h == 0))
                        )
                        early = g == 0 and kc == 1 and (j == 0 or (j == 1 and h == 0))
                        mid = g + 3 < n_groups and kc == 1 and (
                            (j in (1, 2) and h == 1) or (j in (1, 2) and h == 0)
                        )
                        stash_this = late or early or mid
                        dst_ap = out_r[:, bs[j], kc, 2 * h * NW : 2 * h * NW + 2 * NW]
                        if stash_this:
                            po = stashpool.tile(
                                [128, 2, NW], BF16, tag="stash", name="stash"
                            )
                            if late and j == 1:
                                # sprinkled into the last group's store stream
                                # on SP (instantly ready there), bypassing the
                                # Pool queue's ~1us/piece desc-gen cadence
                                late_sp.append((po, dst_ap))
                            else:
                                # stash wave order: early=~g1, mid=g+2, late=g7
                                wave = 1 if early else (g + 2 if mid else n_groups - 1)
                                stash.append((po, dst_ap, wave))
                        else:
                            po = opool.tile([128, 2, NW], BF16, tag="po", name="po")
                        # strictly alternate ACT/DVE: same-engine runs
                        # serialize the 2-buffer o_ps PSUM rotation
                        if ev % 2 == 0:
                            act_copies.append(nc.scalar.copy(po, o_ps))
                        else:
                            dve_copies.append(nc.vector.tensor_copy(po, o_ps))
                        ev += 1
                        if not stash_this:
                            # stream each [128,1024] piece out independently
                            nc.sync.dma_start(out=dst_ap, in_=po)
                            if g == n_groups - 1 and late_sp and ev % 2 == 0:
                                spo, sdst = late_sp.pop(0)
                                nc.sync.dma_start(out=sdst, in_=spo)

                if g + 4 < n_groups:
                    load_w(group_bs(g + 4)[2 * h + 1])

            prev_dve_copies, prev_act_copies = dve_copies, act_copies

        # deferred stash stores on the idle Pool/SWDGE queue.  The early wave
        # is gated on group 1's second-half matmuls (fills the load-burst ->
        # steady-state lull); the late wave on the last group's first matmul
        # (fills the tail while the final group's stores trickle through its
        # compute chain)
        for po, dst, wave in sorted(stash, key=lambda s: s[2]):
            d = nc.gpsimd.dma_start(out=dst, in_=po)
            gate = gate2_inst if wave == 1 else first_mm1[wave]
            tile.add_dep_helper(
                d.ins, gate.ins, sync=True,
                reason="defer stash stores into DMA lulls",
            )

    nc.compile()
    return nc


_NC_CACHE = {}


def _get_nc(n_batch=B):
    if n_batch not in _NC_CACHE:
        _NC_CACHE[n_batch] = build_nc(n_batch)
    return _NC_CACHE[n_batch]


def prep_features(features):
    """[nb, C, H, W] f32 -> padded bf16 [128, KC, nb, 32], pre-scaled 1/WS."""
    features = np.asarray(features, dtype=np.float32) * (1.0 / WS)
    nb = features.shape[0]
    f4 = features.reshape(nb, KC, 128, HW)
    fpad = np.zeros((nb, KC, 128, 32), np.float32)
    fpad[..., :HW] = f4
    return np.ascontiguousarray(fpad.transpose(2, 1, 0, 3)).astype(BF_NP)


def run(features, weights, trace=False, **kwargs):
    """Shard over 8 cores, run, gather. Returns (out, BassKernelResults)."""
    fpad = prep_features(features)
    weights = (np.asarray(weights, dtype=np.float32) * WS).astype(
        ml_dtypes.float8_e3m4
    )
    aux = aux_inputs()
    nc = _get_nc()
    in_maps = []
    for i in range(N_CORES):
        sl = slice(i * B, (i + 1) * B)
        in_maps.append({"fpad": fpad[:, :, sl], "weights": weights[sl], **aux})
    res = run_bass_kernel_spmd(
        nc, in_maps, core_ids=list(range(N_CORES)), trace=trace, **kwargs
    )
    out = np.concatenate(
        [np.asarray(r["out"]).astype(np.float32) for r in res.results], axis=0
    )
    return out, res


def kernel(features, weights):
    out, _ = run(features, weights)
    return out


# revision 101
# speedup vs baseline: 1.0066x; 1.0066x over previous
"""Attentional pooling layer on Trainium2 (Bass/Tile), 8-core batch-parallel.

Reference computation per batch b:
    scores[hw, n] = sum_c f[c, hw] * w[c, n]          (mm1)
    num           = softplus(scores)                  (ACT: Abs/Exp/Ln)
    denom[n]      = sum_hw num[hw, n]                 (PE reduce)
    att[hw, n]    = num / denom[n]                    (DVE recip + mult)
    out[c, n]     = sum_hw f[c, hw] * att[hw, n]      (mm2)

HBM time is the bottleneck, so bytes are minimized against the rel-err
budget (2e-2): weights travel as fp8 e3m4 (see WS below), activations and
outputs as bf16.  Measured end-to-end rel err: 1.49e-2 (seed-fixed,
deterministic).  The +1e-4 numerator / +16e-4 denominator constants of the
reference are dropped entirely (measured end-to-end impact 5.5e-5).

Partition layout: 4 batches per group at 32-partition offsets 0/32/64/96
(explicit tile_position, bypassing the conservative 0/32/64 helper assert;
base 96 verified on hardware).  32 batches per core = 8 exact groups.

The denominator reduce-over-hw AND its broadcast back to all 128
partitions are fused into ONE matmul with a constant 0/1 matrix
C[k, m] = 1 iff k, m in the same 32-block and k%32 < 16.  A single act
table (natural_log_exp_and_others: Abs+Exp+Ln+Copy) serves every ACT op,
so exactly one table load is emitted — forced by blanking the other table
entries (indices preserved, so act_func_set_id still matches walrus's
act_info.json).

Output drain PSUM->SBUF casts f32->bf16 on DVE/ACT (GPSIMD cannot touch
PSUM), in [128,1024] units (2 PSUM banks) to amortize access latency.
"""

import numpy as np
import ml_dtypes
from contextlib import ExitStack

import concourse.bass as bass
import concourse.bacc as bacc
import concourse.tile as tile
from concourse import mybir
from concourse.bass_utils import run_bass_kernel_spmd

F32 = mybir.dt.float32
BF16 = mybir.dt.bfloat16
F8E3 = mybir.dt.float8e3
AF = mybir.ActivationFunctionType
ALU = mybir.AluOpType
BF_NP = ml_dtypes.bfloat16

# Weights travel as fp8 e3m4 (1 byte), halving the weight-load traffic again.
# The PE multiplies fp8 operands exactly, so only the quantization of w
# matters: end-to-end rel err 1.50e-2 vs the 2e-2 gate (measured, seed-fixed,
# deterministic).  w is pre-scaled by WS=2.5 into e3m4's [~0.008, 15.5]
# band; features carry 1/WS so scores come out exact, and the denominator
# matrix carries 1/WS so mm2's (f/WS) @ (WS*att) cancels exactly.
WS = 2.5

# Force every ACT op onto one table so only a single table load is emitted.
# Keys/positions are preserved (ids index walrus's act_info.json); only the
# *selection* changes: all other tables are presented as empty so the greedy
# insert pass picks natural_log_exp_and_others (Abs+Exp+Ln+Copy) for all.
_KEEP_TABLE = "natural_log_exp_and_others"
_orig_get_tables = bacc.get_activation_tables


def _single_table(arch):
    tbls = _orig_get_tables(arch)
    return {k: (v if k == _KEEP_TABLE else set()) for k, v in tbls.items()}


bacc.get_activation_tables = _single_table

N_CORES = 8
B_FULL, C, H, W, N = 256, 256, 4, 4, 2048
HW = H * W                  # 16
B = B_FULL // N_CORES       # 32 batches per core
KC = C // 128               # 2 contraction chunks of 128
GB = 4                      # batches per partition group (offsets 0/32/64/96)
GP = 32 * GB                # 128 partitions per group
NCH = 4                     # n chunks per group
NW = N // NCH               # 512 (one PSUM bank)


def aux_inputs():
    # aux[:, 0, :] = cmat: cmat[k, m] = 1 iff k and m share a 32-block and
    # k%32 is a real hw row; (C^T @ num) both reduces over hw and broadcasts
    # back to 128 partitions.  aux[:, 1, :] = 128x128 identity (transposes).
    cmat = np.zeros((GP, GP), np.float32)
    for k in range(GP):
        for m in range(GP):
            if k // 32 == m // 32 and k % 32 < HW:
                cmat[k, m] = 1.0 / WS
    aux = np.stack([cmat, np.eye(128, dtype=np.float32)], axis=1)
    return {"aux": aux.astype(BF_NP)}


def build_nc(n_batch=B, debug=False):
    nc = bacc.Bacc(None, target_bir_lowering=False, debug=debug)
    feat = nc.dram_tensor("fpad", [128, KC, n_batch, 32], BF16, kind="ExternalInput")
    wts = nc.dram_tensor("weights", [n_batch, C, N], F8E3, kind="ExternalInput")
    out = nc.dram_tensor("out", [n_batch, C, N], BF16, kind="ExternalOutput")
    aux_d = nc.dram_tensor("aux", [128, 2, 128], BF16, kind="ExternalInput")

    # [ci, b, kc, n] views of the DRAM tensors
    wts_r = wts.ap().rearrange("b (kc ci) n -> ci b kc n", kc=KC)
    out_r = out.ap().rearrange("b (kc ci) n -> ci b kc n", kc=KC)

    n_groups = (n_batch + GB - 1) // GB

    with tile.TileContext(nc) as tc, ExitStack() as ctx:
        singles = ctx.enter_context(tc.tile_pool(name="singles", bufs=1))
        wpool = ctx.enter_context(tc.tile_pool(name="w", bufs=18))
        opool = ctx.enter_context(tc.tile_pool(name="o", bufs=6))
        stashpool = ctx.enter_context(tc.tile_pool(name="stash", bufs=44))
        stash = []
        late_sp = []
        gate_inst = None
        gate2_inst = None
        first_mm1 = {}
        sppool = ctx.enter_context(tc.tile_pool(name="sp", bufs=2))
        numpool = ctx.enter_context(tc.tile_pool(name="num", bufs=2))
        rpool = ctx.enter_context(tc.tile_pool(name="r", bufs=2))
        attpool = ctx.enter_context(tc.tile_pool(name="att", bufs=2))
        ftpool = ctx.enter_context(tc.tile_pool(name="ft", bufs=2))
        ps_o = ctx.enter_context(tc.tile_pool(name="ps_o", bufs=2, space="PSUM"))
        chunk_psum = ExitStack()
        ps_sc = chunk_psum.enter_context(tc.tile_pool(name="ps_sc", bufs=2, space="PSUM"))
        ps_d = chunk_psum.enter_context(tc.tile_pool(name="ps_d", bufs=1, space="PSUM"))
        ps_ft = chunk_psum.enter_context(tc.tile_pool(name="ps_ft", bufs=1, space="PSUM"))
        ps_o2 = None

        # features, pre-transposed + hw-padded to 32 with zeros on the host
        f_t = singles.tile([128, KC, n_batch, 32], BF16)
        nc.sync.dma_start(out=f_t, in_=feat.ap())

        aux_t = singles.tile([128, 2, 128], BF16)
        nc.sync.dma_start(out=aux_t, in_=aux_d.ap())
        cm_t = aux_t[:, 0, :]
        id_t = aux_t[:, 1, :]

        def group_bs(gg):
            return [min(gg * GB + j, n_batch - 1) for j in range(GB)]

        # W loads are issued two groups ahead of use, so on the SP queue they
        # sit in front of the piece stores of the group being computed —
        # otherwise those stores' copy-waits head-of-line-block ready loads
        # and starve the DMA engines
        w_t = {}

        def load_w(b):
            if b not in w_t:
                w_t[b] = wpool.tile([128, KC, N], F8E3, tag="w", name="w_t")
                nc.sync.dma_start(out=w_t[b], in_=wts_r[:, b])

        def load_group_w(gg):
            if gg < n_groups:
                for b in sorted(set(group_bs(gg))):
                    load_w(b)

        load_group_w(0)
        load_group_w(1)
        load_group_w(2)
        load_group_w(3)

        ev = 0
        prev_dve_copies = []
        prev_act_copies = []
        for g in range(n_groups):
            bs = group_bs(g)
            emit = [g * GB + j < n_batch for j in range(GB)]
            dve_copies, act_copies = [], []

            # transposed features fT[hw, c] for mm2, one 128x128 transpose
            # per kc (slot j's fT lands at partition 32j automatically)
            ft_ps = ps_ft.tile([GP, KC, 128], BF16, name="ft_ps")
            for kc in range(KC):
                nc.tensor.transpose(
                    ft_ps[:, kc, :],
                    f_t[:, kc, bs[0] : bs[0] + GB, :],
                    id_t,
                )
            ft_sb = ftpool.tile([GP, KC, 128], BF16, name="ft_sb")
            nc.scalar.copy(ft_sb, ft_ps)

            # two halves of 2 n-chunks each: mm2 for a half only needs that
            # half's att chunks, so output stores stream mid-group instead of
            # bunching at the group end (shrinks the tail and group-boundary
            # DMA droughts)
            for h in range(2):
                # issue 2 of group g+2's weight loads at the top of each
                # half: the SP queue then alternates [2 loads, ~7 stores],
                # and each half's chunk-compute lull is covered by the loads
                # issued just ahead of the stores
                if g + 4 < n_groups:
                    bs2 = group_bs(g + 4)
                    load_w(bs2[2 * h])
                att_h = attpool.tile([GP, 2, NW], BF16, tag="att")
                for q in range(2):
                    nb = 2 * h + q
                    sc_ps = ps_sc.tile([GP, NW], F32, name="sc_ps")
                    for j in range(GB):
                        for kc in range(KC):
                            mm = nc.tensor.matmul(
                                sc_ps[32 * j : 32 * j + 32, :],
                                f_t[:, kc, bs[j], :],
                                w_t[bs[j]][:, kc, nb * NW : (nb + 1) * NW],
                                start=(kc == 0),
                                stop=(kc == KC - 1),
                                tile_position=(0, 32 * j),
                            )
                            if g == n_groups - 1 and gate_inst is None:
                                gate_inst = mm
                            if g == 1 and h == 1 and gate2_inst is None:
                                gate2_inst = mm
                            if h == 0 and g not in first_mm1:
                                first_mm1[g] = mm
                    # softplus(x) = max(x,0) + ln(1 + exp(-|x|)): exp arg <= 0
                    # so no overflow, Ln input stays in [1,2]
                    t_abs = sppool.tile([GP, NW], F32, tag="tabs")
                    nc.scalar.activation(t_abs, sc_ps, AF.Abs)
                    t_exp = sppool.tile([GP, NW], F32, tag="texp")
                    nc.scalar.activation(t_exp, t_abs, AF.Exp, scale=-1.0)
                    t_ln = sppool.tile([GP, NW], F32, tag="tln")
                    nc.scalar.activation(t_ln, t_exp, AF.Ln, bias=1.0)
                    num_t = numpool.tile([GP, NW], BF16, tag="num")
                    nc.vector.scalar_tensor_tensor(
                        num_t, sc_ps, 0.0, t_ln, op0=ALU.max, op1=ALU.add
                    )
                    # fused reduce-over-hw + broadcast:
                    #   d[m] = sum_k C[k,m] num[k]
                    d_ps = ps_d.tile([GP, NW], F32, name="d_ps")
                    nc.tensor.matmul(d_ps, cm_t, num_t, start=True, stop=True)
                    r_t = rpool.tile([GP, NW], F32, tag="r")
                    nc.vector.reciprocal(r_t, d_ps)
                    # att = num * (1/d): all-SBUF, so it can run on the
                    # otherwise-idle GPSIMD engine instead of DVE
                    nc.gpsimd.tensor_tensor(
                        att_h[:, q, :], num_t, r_t, op=ALU.mult
                    )

                if g == n_groups - 1 and h == 1:
                    # the chunk pipeline is finished for good: release its 4
                    # PSUM banks and re-use them as 2 extra output buffers,
                    # halving the tail's copy->mm2 rotation cycle
                    chunk_psum.close()
                    ps_o2 = ctx.enter_context(
                        tc.tile_pool(name="ps_o2", bufs=2, space="PSUM")
                    )
                for j in range(GB):
                    if not emit[j]:
                        continue
                    for kc in range(KC):
                        opool_ps = ps_o2 if (ps_o2 is not None and ev % 2) else ps_o
                        o_ps = opool_ps.tile([128, 2, NW], F32)
                        for q in range(2):
                            nc.tensor.matmul(
                                o_ps[:, q, :],
                                ft_sb[32 * j : 32 * j + HW, kc, :],
                                att_h[32 * j : 32 * j + HW, q, :],
                                start=True,
                                stop=True,
                                tile_position=(32 * j, 0),
                            )
                        # stash some pieces: their DMAs are deferred into DMA
                        # lulls.  late wave -> the tail; early wave -> the
                        # t~50us load-burst lull; mid waves -> the recurring
                        # group-boundary lulls two groups later
                        late = (
                            g < n_groups - 1 and kc == 0 and (j == 0 or (j == 1 and h == 0))
                        )
                        early = g == 0 and kc == 1 and (j == 0 or (j == 1 and h == 0))
                        mid = g + 3 < n_groups and kc == 1 and (
                            (j in (1, 2) and h == 1) or (j in (1, 2) and h == 0)
                        )
                        stash_this = late or early or mid
                        dst_ap = out_r[:, bs[j], kc, 2 * h * NW : 2 * h * NW + 2 * NW]
                        if stash_this:
                            po = stashpool.tile(
                                [128, 2, NW], BF16, tag="stash", name="stash"
                            )
                            if late and j == 1:
                                # sprinkled into the last group's store stream
                                # on SP (instantly ready there), bypassing the
                                # Pool queue's ~1us/piece desc-gen cadence
                                late_sp.append((po, dst_ap))
                            else:
                                # stash wave order: early=~g1, mid=g+2, late=g7
                                wave = 1 if early else (g + 2 if mid else n_groups - 1)
                                stash.append((po, dst_ap, wave))
                        else:
                            po = opool.tile([128, 2, NW], BF16, tag="po", name="po")
                        # strictly alternate ACT/DVE: same-engine runs
                        # serialize the 2-buffer o_ps PSUM rotation
                        if ev % 2 == 0:
                            act_copies.append(nc.scalar.copy(po, o_ps))
                        else:
                            dve_copies.append(nc.vector.tensor_copy(po, o_ps))
                        ev += 1
                        if not stash_this:
                            # stream each [128,1024] piece out independently
                            nc.sync.dma_start(out=dst_ap, in_=po)
                            if g == n_groups - 1 and late_sp and ev % 2 == 0:
                                spo, sdst = late_sp.pop(0)
                                nc.sync.dma_start(out=sdst, in_=spo)

                if g + 4 < n_groups:
                    load_w(group_bs(g + 4)[2 * h + 1])

            prev_dve_copies, prev_act_copies = dve_copies, act_copies

        # deferred stash stores on the idle Pool/SWDGE queue.  The early wave
        # is gated on group 1's second-half matmuls (fills the load-burst ->
        # steady-state lull); the late wave on the last group's first matmul
        # (fills the tail while the final group's stores trickle through its
        # compute chain)
        for po, dst, wave in sorted(stash, key=lambda s: s[2]):
            d = nc.gpsimd.dma_start(out=dst, in_=po)
            gate = gate2_inst if wave == 1 else first_mm1[wave]
            tile.add_dep_helper(
                d.ins, gate.ins, sync=True,
                reason="defer stash stores into DMA lulls",
            )

    nc.compile()
    return nc


_NC_CACHE = {}


def _get_nc(n_batch=B):
    if n_batch not in _NC_CACHE:
        _NC_CACHE[n_batch] = build_nc(n_batch)
    return _NC_CACHE[n_batch]


def prep_features(features):
    """[nb, C, H, W] f32 -> padded bf16 [128, KC, nb, 32], pre-scaled 1/WS."""
    features = np.asarray(features, dtype=np.float32) * (1.0 / WS)
    nb = features.shape[0]
    f4 = features.reshape(nb, KC, 128, HW)
    fpad = np.zeros((nb, KC, 128, 32), np.float32)
    fpad[..., :HW] = f4
    return np.ascontiguousarray(fpad.transpose(2, 1, 0, 3)).astype(BF_NP)


def run(features, weights, trace=False, **kwargs):
    """Shard over 8 cores, run, gather. Returns (out, BassKernelResults)."""
    fpad = prep_features(features)
    weights = (np.asarray(weights, dtype=np.float32) * WS).astype(
        ml_dtypes.float8_e3m4
    )
    aux = aux_inputs()
    nc = _get_nc()
    in_maps = []
    for i in range(N_CORES):
        sl = slice(i * B, (i + 1) * B)
        in_maps.append({"fpad": fpad[:, :, sl], "weights": weights[sl], **aux})
    res = run_bass_kernel_spmd(
        nc, in_maps, core_ids=list(range(N_CORES)), trace=trace, **kwargs
    )
    out = np.concatenate(
        [np.asarray(r["out"]).astype(np.float32) for r in res.results], axis=0
    )
    return out, res


def kernel(features, weights):
    out, _ = run(features, weights)
    return out


# revision 104
# speedup vs baseline: 1.0071x; 1.0005x over previous
"""Attentional pooling layer on Trainium2 (Bass/Tile), 8-core batch-parallel.

Reference computation per batch b:
    scores[hw, n] = sum_c f[c, hw] * w[c, n]          (mm1)
    num           = softplus(scores)                  (ACT: Abs/Exp/Ln)
    denom[n]      = sum_hw num[hw, n]                 (PE reduce)
    att[hw, n]    = num / denom[n]                    (DVE recip + mult)
    out[c, n]     = sum_hw f[c, hw] * att[hw, n]      (mm2)

HBM time is the bottleneck, so bytes are minimized against the rel-err
budget (2e-2): weights travel as fp8 e3m4 (see WS below), activations and
outputs as bf16.  Measured end-to-end rel err: 1.49e-2 (seed-fixed,
deterministic).  The +1e-4 numerator / +16e-4 denominator constants of the
reference are dropped entirely (measured end-to-end impact 5.5e-5).

Partition layout: 4 batches per group at 32-partition offsets 0/32/64/96
(explicit tile_position, bypassing the conservative 0/32/64 helper assert;
base 96 verified on hardware).  32 batches per core = 8 exact groups.

The denominator reduce-over-hw AND its broadcast back to all 128
partitions are fused into ONE matmul with a constant 0/1 matrix
C[k, m] = 1 iff k, m in the same 32-block and k%32 < 16.  A single act
table (natural_log_exp_and_others: Abs+Exp+Ln+Copy) serves every ACT op,
so exactly one table load is emitted — forced by blanking the other table
entries (indices preserved, so act_func_set_id still matches walrus's
act_info.json).

Output drain PSUM->SBUF casts f32->bf16 on DVE/ACT (GPSIMD cannot touch
PSUM), in [128,1024] units (2 PSUM banks) to amortize access latency.
"""

import numpy as np
import ml_dtypes
from contextlib import ExitStack

import concourse.bass as bass
import concourse.bacc as bacc
import concourse.tile as tile
from concourse import mybir
from concourse.bass_utils import run_bass_kernel_spmd

F32 = mybir.dt.float32
BF16 = mybir.dt.bfloat16
F8E3 = mybir.dt.float8e3
AF = mybir.ActivationFunctionType
ALU = mybir.AluOpType
BF_NP = ml_dtypes.bfloat16

# Weights travel as fp8 e3m4 (1 byte), halving the weight-load traffic again.
# The PE multiplies fp8 operands exactly, so only the quantization of w
# matters: end-to-end rel err 1.50e-2 vs the 2e-2 gate (measured, seed-fixed,
# deterministic).  w is pre-scaled by WS=2.5 into e3m4's [~0.008, 15.5]
# band; features carry 1/WS so scores come out exact, and the denominator
# matrix carries 1/WS so mm2's (f/WS) @ (WS*att) cancels exactly.
WS = 2.5

# Force every ACT op onto one table so only a single table load is emitted.
# Keys/positions are preserved (ids index walrus's act_info.json); only the
# *selection* changes: all other tables are presented as empty so the greedy
# insert pass picks natural_log_exp_and_others (Abs+Exp+Ln+Copy) for all.
_KEEP_TABLE = "natural_log_exp_and_others"
_orig_get_tables = bacc.get_activation_tables


def _single_table(arch):
    tbls = _orig_get_tables(arch)
    return {k: (v if k == _KEEP_TABLE else set()) for k, v in tbls.items()}


bacc.get_activation_tables = _single_table

N_CORES = 8
B_FULL, C, H, W, N = 256, 256, 4, 4, 2048
HW = H * W                  # 16
B = B_FULL // N_CORES       # 32 batches per core
KC = C // 128               # 2 contraction chunks of 128
GB = 4                      # batches per partition group (offsets 0/32/64/96)
GP = 32 * GB                # 128 partitions per group
NCH = 4                     # n chunks per group
NW = N // NCH               # 512 (one PSUM bank)


def aux_inputs():
    # aux[:, 0, :] = cmat: cmat[k, m] = 1 iff k and m share a 32-block and
    # k%32 is a real hw row; (C^T @ num) both reduces over hw and broadcasts
    # back to 128 partitions.  aux[:, 1, :] = 128x128 identity (transposes).
    cmat = np.zeros((GP, GP), np.float32)
    for k in range(GP):
        for m in range(GP):
            if k // 32 == m // 32 and k % 32 < HW:
                cmat[k, m] = 1.0 / WS
    aux = np.stack([cmat, np.eye(128, dtype=np.float32)], axis=1)
    return {"aux": aux.astype(BF_NP)}


def build_nc(n_batch=B, debug=False):
    nc = bacc.Bacc(None, target_bir_lowering=False, debug=debug)
    feat = nc.dram_tensor("fpad", [128, KC, n_batch, 32], BF16, kind="ExternalInput")
    wts = nc.dram_tensor("weights", [n_batch, C, N], F8E3, kind="ExternalInput")
    out = nc.dram_tensor("out", [n_batch, C, N], BF16, kind="ExternalOutput")
    aux_d = nc.dram_tensor("aux", [128, 2, 128], BF16, kind="ExternalInput")

    # [ci, b, kc, n] views of the DRAM tensors
    wts_r = wts.ap().rearrange("b (kc ci) n -> ci b kc n", kc=KC)
    out_r = out.ap().rearrange("b (kc ci) n -> ci b kc n", kc=KC)

    n_groups = (n_batch + GB - 1) // GB

    with tile.TileContext(nc) as tc, ExitStack() as ctx:
        singles = ctx.enter_context(tc.tile_pool(name="singles", bufs=1))
        wpool = ctx.enter_context(tc.tile_pool(name="w", bufs=18))
        opool = ctx.enter_context(tc.tile_pool(name="o", bufs=9))
        stashpool = ctx.enter_context(tc.tile_pool(name="stash", bufs=44))
        stash = []
        late_sp = []
        gate_inst = None
        gate2_inst = None
        first_mm1 = {}
        sppool = ctx.enter_context(tc.tile_pool(name="sp", bufs=2))
        numpool = ctx.enter_context(tc.tile_pool(name="num", bufs=2))
        rpool = ctx.enter_context(tc.tile_pool(name="r", bufs=2))
        attpool = ctx.enter_context(tc.tile_pool(name="att", bufs=2))
        ftpool = ctx.enter_context(tc.tile_pool(name="ft", bufs=2))
        ps_o = ctx.enter_context(tc.tile_pool(name="ps_o", bufs=2, space="PSUM"))
        chunk_psum = ExitStack()
        ps_sc = chunk_psum.enter_context(tc.tile_pool(name="ps_sc", bufs=2, space="PSUM"))
        ps_d = chunk_psum.enter_context(tc.tile_pool(name="ps_d", bufs=1, space="PSUM"))
        ps_ft = chunk_psum.enter_context(tc.tile_pool(name="ps_ft", bufs=1, space="PSUM"))
        ps_o2 = None

        # features, pre-transposed + hw-padded to 32 with zeros on the host
        f_t = singles.tile([128, KC, n_batch, 32], BF16)
        nc.sync.dma_start(out=f_t, in_=feat.ap())

        aux_t = singles.tile([128, 2, 128], BF16)
        nc.sync.dma_start(out=aux_t, in_=aux_d.ap())
        cm_t = aux_t[:, 0, :]
        id_t = aux_t[:, 1, :]

        def group_bs(gg):
            return [min(gg * GB + j, n_batch - 1) for j in range(GB)]

        # W loads are issued two groups ahead of use, so on the SP queue they
        # sit in front of the piece stores of the group being computed —
        # otherwise those stores' copy-waits head-of-line-block ready loads
        # and starve the DMA engines
        w_t = {}

        def load_w(b):
            if b not in w_t:
                w_t[b] = wpool.tile([128, KC, N], F8E3, tag="w", name="w_t")
                nc.sync.dma_start(out=w_t[b], in_=wts_r[:, b])

        def load_group_w(gg):
            if gg < n_groups:
                for b in sorted(set(group_bs(gg))):
                    load_w(b)

        load_group_w(0)
        load_group_w(1)
        load_group_w(2)
        load_group_w(3)

        ev = 0
        prev_dve_copies = []
        prev_act_copies = []
        for g in range(n_groups):
            bs = group_bs(g)
            emit = [g * GB + j < n_batch for j in range(GB)]
            dve_copies, act_copies = [], []

            # transposed features fT[hw, c] for mm2, one 128x128 transpose
            # per kc (slot j's fT lands at partition 32j automatically)
            ft_ps = ps_ft.tile([GP, KC, 128], BF16, name="ft_ps")
            for kc in range(KC):
                nc.tensor.transpose(
                    ft_ps[:, kc, :],
                    f_t[:, kc, bs[0] : bs[0] + GB, :],
                    id_t,
                )
            ft_sb = ftpool.tile([GP, KC, 128], BF16, name="ft_sb")
            nc.scalar.copy(ft_sb, ft_ps)

            # two halves of 2 n-chunks each: mm2 for a half only needs that
            # half's att chunks, so output stores stream mid-group instead of
            # bunching at the group end (shrinks the tail and group-boundary
            # DMA droughts)
            for h in range(2):
                # issue 2 of group g+2's weight loads at the top of each
                # half: the SP queue then alternates [2 loads, ~7 stores],
                # and each half's chunk-compute lull is covered by the loads
                # issued just ahead of the stores
                if g + 4 < n_groups:
                    bs2 = group_bs(g + 4)
                    load_w(bs2[2 * h])
                att_h = attpool.tile([GP, 2, NW], BF16, tag="att")
                for q in range(2):
                    nb = 2 * h + q
                    sc_ps = ps_sc.tile([GP, NW], F32, name="sc_ps")
                    for j in range(GB):
                        for kc in range(KC):
                            mm = nc.tensor.matmul(
                                sc_ps[32 * j : 32 * j + 32, :],
                                f_t[:, kc, bs[j], :],
                                w_t[bs[j]][:, kc, nb * NW : (nb + 1) * NW],
                                start=(kc == 0),
                                stop=(kc == KC - 1),
                                tile_position=(0, 32 * j),
                            )
                            if g == n_groups - 1 and gate_inst is None:
                                gate_inst = mm
                            if g == 1 and h == 1 and gate2_inst is None:
                                gate2_inst = mm
                            if h == 0 and g not in first_mm1:
                                first_mm1[g] = mm
                    # softplus(x) = max(x,0) + ln(1 + exp(-|x|)): exp arg <= 0
                    # so no overflow, Ln input stays in [1,2]
                    t_abs = sppool.tile([GP, NW], F32, tag="tabs")
                    nc.scalar.activation(t_abs, sc_ps, AF.Abs)
                    t_exp = sppool.tile([GP, NW], F32, tag="texp")
                    nc.scalar.activation(t_exp, t_abs, AF.Exp, scale=-1.0)
                    t_ln = sppool.tile([GP, NW], F32, tag="tln")
                    nc.scalar.activation(t_ln, t_exp, AF.Ln, bias=1.0)
                    num_t = numpool.tile([GP, NW], BF16, tag="num")
                    nc.vector.scalar_tensor_tensor(
                        num_t, sc_ps, 0.0, t_ln, op0=ALU.max, op1=ALU.add
                    )
                    # fused reduce-over-hw + broadcast:
                    #   d[m] = sum_k C[k,m] num[k]
                    d_ps = ps_d.tile([GP, NW], F32, name="d_ps")
                    nc.tensor.matmul(d_ps, cm_t, num_t, start=True, stop=True)
                    r_t = rpool.tile([GP, NW], F32, tag="r")
                    nc.vector.reciprocal(r_t, d_ps)
                    # att = num * (1/d): all-SBUF, so it can run on the
                    # otherwise-idle GPSIMD engine instead of DVE
                    nc.gpsimd.tensor_tensor(
                        att_h[:, q, :], num_t, r_t, op=ALU.mult
                    )

                if g == n_groups - 1 and h == 1:
                    # the chunk pipeline is finished for good: release its 4
                    # PSUM banks and re-use them as 2 extra output buffers,
                    # halving the tail's copy->mm2 rotation cycle
                    chunk_psum.close()
                    ps_o2 = ctx.enter_context(
                        tc.tile_pool(name="ps_o2", bufs=2, space="PSUM")
                    )
                for j in range(GB):
                    if not emit[j]:
                        continue
                    for kc in range(KC):
                        opool_ps = ps_o2 if (ps_o2 is not None and ev % 2) else ps_o
                        o_ps = opool_ps.tile([128, 2, NW], F32)
                        for q in range(2):
                            nc.tensor.matmul(
                                o_ps[:, q, :],
                                ft_sb[32 * j : 32 * j + HW, kc, :],
                                att_h[32 * j : 32 * j + HW, q, :],
                                start=True,
                                stop=True,
                                tile_position=(32 * j, 0),
                            )
                        # stash some pieces: their DMAs are deferred into DMA
                        # lulls.  late wave -> the tail; early wave -> the
                        # t~50us load-burst lull; mid waves -> the recurring
                        # group-boundary lulls two groups later
                        late = (
                            g < n_groups - 1 and kc == 0 and (j == 0 or (j == 1 and h == 0))
                        )
                        early = g == 0 and kc == 1 and (j == 0 or (j == 1 and h == 0))
                        mid = g + 3 < n_groups and kc == 1 and (
                            (j in (1, 2) and h == 1) or (j in (1, 2) and h == 0)
                        )
                        stash_this = late or early or mid
                        dst_ap = out_r[:, bs[j], kc, 2 * h * NW : 2 * h * NW + 2 * NW]
                        if stash_this:
                            po = stashpool.tile(
                                [128, 2, NW], BF16, tag="stash", name="stash"
                            )
                            if late and j == 1:
                                # sprinkled into the last group's store stream
                                # on SP (instantly ready there), bypassing the
                                # Pool queue's ~1us/piece desc-gen cadence
                                late_sp.append((po, dst_ap))
                            else:
                                # stash wave order: early=~g1, mid=g+2, late=g7
                                wave = 1 if early else (g + 2 if mid else n_groups - 1)
                                stash.append((po, dst_ap, wave))
                        else:
                            po = opool.tile([128, 2, NW], BF16, tag="po", name="po")
                        # strictly alternate ACT/DVE: same-engine runs
                        # serialize the 2-buffer o_ps PSUM rotation
                        if ev % 2 == 0:
                            act_copies.append(nc.scalar.copy(po, o_ps))
                        else:
                            dve_copies.append(nc.vector.tensor_copy(po, o_ps))
                        ev += 1
                        if not stash_this:
                            # stream each [128,1024] piece out independently
                            nc.sync.dma_start(out=dst_ap, in_=po)
                            if g == n_groups - 1 and late_sp and ev % 2 == 0:
                                spo, sdst = late_sp.pop(0)
                                nc.sync.dma_start(out=sdst, in_=spo)

                if g + 4 < n_groups:
                    load_w(group_bs(g + 4)[2 * h + 1])

            prev_dve_copies, prev_act_copies = dve_copies, act_copies

        # deferred stash stores on the idle Pool/SWDGE queue.  The early wave
        # is gated on group 1's second-half matmuls (fills the load-burst ->
        # steady-state lull); the late wave on the last group's first matmul
        # (fills the tail while the final group's stores trickle through its
        # compute chain)
        for po, dst, wave in sorted(stash, key=lambda s: s[2]):
            d = nc.gpsimd.dma_start(out=dst, in_=po)
            gate = gate2_inst if wave == 1 else first_mm1[wave]
            tile.add_dep_helper(
                d.ins, gate.ins, sync=True,
                reason="defer stash stores into DMA lulls",
            )

    nc.compile()
    return nc


_NC_CACHE = {}


def _get_nc(n_batch=B):
    if n_batch not in _NC_CACHE:
        _NC_CACHE[n_batch] = build_nc(n_batch)
    return _NC_CACHE[n_batch]


def prep_features(features):
    """[nb, C, H, W] f32 -> padded bf16 [128, KC, nb, 32], pre-scaled 1/WS."""
    features = np.asarray(features, dtype=np.float32) * (1.0 / WS)
    nb = features.shape[0]
    f4 = features.reshape(nb, KC, 128, HW)
    fpad = np.zeros((nb, KC, 128, 32), np.float32)
    fpad[..., :HW] = f4
    return np.ascontiguousarray(fpad.transpose(2, 1, 0, 3)).astype(BF_NP)


def run(features, weights, trace=False, **kwargs):
    """Shard over 8 cores, run, gather. Returns (out, BassKernelResults)."""
    fpad = prep_features(features)
    weights = (np.asarray(weights, dtype=np.float32) * WS).astype(
        ml_dtypes.float8_e3m4
    )
    aux = aux_inputs()
    nc = _get_nc()
    in_maps = []
    for i in range(N_CORES):
        sl = slice(i * B, (i + 1) * B)
        in_maps.append({"fpad": fpad[:, :, sl], "weights": weights[sl], **aux})
    res = run_bass_kernel_spmd(
        nc, in_maps, core_ids=list(range(N_CORES)), trace=trace, **kwargs
    )
    out = np.concatenate(
        [np.asarray(r["out"]).astype(np.float32) for r in res.results], axis=0
    )
    return out, res


def kernel(features, weights):
    out, _ = run(features, weights)
    return out
